# revision 1
# baseline (speedup 1.0000x reference)
"""TRN2 Bass kernel for nn_BiDecoder (GNN edge rating decoder), 8 NeuronCores.

ratings[e] = sum_r softmax_r(ufeat[src[e]] @ Ps[r] @ ifeat[dst[e]]) * (r+1)

Sharding: edges sorted by dst -> 8 contiguous shards (each core sees a narrow
item band, gathered with int16 indices from a per-core ifeat slice); within a
shard edges are re-sorted by src so each 8192-edge gather block spans a small
user range (int16 indices against a static 32768-row window of ufeat).
On-device per block: two dma_gathers; per pair of 128-edge tiles a PE
transpose + block-diagonal PsAll matmul produce Z = us @ Ps_all in PSUM; DVE
does B = Z * vs and the f-reduction; batched softmax-weighted sum -> ratings.
"""
import sys

sys.path.insert(0, "/opt/trn_rl_repo")
import numpy as np

P = 128
D = 64
R = 5
N_USERS, N_ITEMS, E = 100000, 50000, 1000000
N_CORES = 8
E_CORE = E // N_CORES
BLK = 8192
N_BLK = (E_CORE + BLK - 1) // BLK
PAD_E = N_BLK * BLK
TILES_PER_BLK = BLK // P
PAIRS_PER_BLK = TILES_PER_BLK // 2
WIN = 32768
IF_ROWS = 8192
S_BLK = BLK // 16
GBLK = 1024  # idxs per dma_gather instruction (16-engine x 64-desc packet limit)

_WIN_OFFS = [
    max(0, min(N_USERS - WIN, int((b + 0.5) * BLK / E_CORE * N_USERS) - WIN // 2))
    for b in range(N_BLK)
]

_NC_CACHE = {}


def _build_kernel():
    import concourse.bacc as bacc
    import concourse.mybir as mybir
    import concourse.tile as tile
    from concourse import library_config
    from concourse.masks import make_identity

    nc = bacc.Bacc(None, target_bir_lowering=False)
    f32, i16 = mybir.dt.float32, mybir.dt.int16

    ufeat_d = nc.dram_tensor("ufeat", [N_USERS, D], f32, kind="ExternalInput")
    ifeat_d = nc.dram_tensor("ifeat_c", [IF_ROWS, D], f32, kind="ExternalInput")
    psall2_d = nc.dram_tensor("psall2", [P, 2 * R * D], f32, kind="ExternalInput")
    idxu_d = nc.dram_tensor("idxu", [P, N_BLK * S_BLK], i16, kind="ExternalInput")
    idxv_d = nc.dram_tensor("idxv", [P, N_BLK * S_BLK], i16, kind="ExternalInput")
    vals_d = nc.dram_tensor("vals", [P, R], f32, kind="ExternalInput")
    out_d = nc.dram_tensor(
        "out", [P, N_BLK * TILES_PER_BLK], f32, kind="ExternalOutput"
    )

    RD = R * D

    with tile.TileContext(nc) as tc:
        nc.gpsimd.load_library(library_config.mlp)
        with (
            tc.tile_pool(name="const", bufs=1) as cpool,
            tc.tile_pool(name="gather", bufs=2) as gpool,
            tc.tile_pool(name="work", bufs=2) as wpool,
            tc.tile_pool(name="psum_z", bufs=2, space="PSUM") as zpool,
            tc.tile_pool(name="psum_t", bufs=2, space="PSUM") as tpool,
        ):
            ident = cpool.tile([P, P], f32)
            make_identity(nc, ident[:])
            psall2 = cpool.tile([P, 2 * RD], f32)
            nc.sync.dma_start(psall2[:], psall2_d[:])
            vals_t = cpool.tile([P, R], f32)
            nc.sync.dma_start(vals_t[:], vals_d[:])
            idxu = cpool.tile([P, N_BLK * S_BLK], i16)
            idxv = cpool.tile([P, N_BLK * S_BLK], i16)
            nc.sync.dma_start(idxu[:], idxu_d[:])
            nc.sync.dma_start(idxv[:], idxv_d[:])

            for b in range(N_BLK):
                ev = min(BLK, E_CORE - b * BLK)          # valid edges
                nt = min(TILES_PER_BLK, -(-ev // P))      # valid tiles
                npair = -(-nt // 2)
                nt = npair * 2
                nsub = -(-(nt * P) // GBLK)               # subgathers needed
                us_g = gpool.tile([P, TILES_PER_BLK * D], f32, tag="us")
                vs_g = gpool.tile([P, TILES_PER_BLK * D], f32, tag="vs")
                w = _WIN_OFFS[b]
                for g in range(nsub):
                    s0 = b * S_BLK + g * (GBLK // 16)
                    s1 = s0 + GBLK // 16
                    c0 = g * (GBLK // P) * D
                    c1 = c0 + (GBLK // P) * D
                    nc.gpsimd.dma_gather(
                        out_ap=us_g[:, c0:c1].rearrange("p (t d) -> p t d", d=D),
                        in_ap=ufeat_d[w : w + WIN, :],
                        idxs_ap=idxu[:, s0:s1],
                        num_idxs=GBLK,
                        num_idxs_reg=GBLK,
                        elem_size=D,
                    )
                    nc.gpsimd.dma_gather(
                        out_ap=vs_g[:, c0:c1].rearrange("p (t d) -> p t d", d=D),
                        in_ap=ifeat_d[:],
                        idxs_ap=idxv[:, s0:s1],
                        num_idxs=GBLK,
                        num_idxs_reg=GBLK,
                        elem_size=D,
                    )

                scores = wpool.tile([P, TILES_PER_BLK * R], f32, tag="scores")
                for pair in range(npair):
                    z_ps = zpool.tile([P, 2, 512], f32, tag="z")
                    uT_ps = tpool.tile([P, P], f32, tag="uT")
                    nc.tensor.transpose(
                        out=uT_ps[:],
                        in_=us_g[:, pair * 2 * D : (pair * 2 + 2) * D],
                        identity=ident[:],
                    )
                    uT_sb = wpool.tile([P, P], f32, tag="uT_sb")
                    nc.scalar.copy(uT_sb[:], uT_ps[:])
                    nc.tensor.matmul(
                        z_ps[:, 0, 0:RD], lhsT=uT_sb[:], rhs=psall2[:, 0:RD]
                    )
                    nc.tensor.matmul(
                        z_ps[:, 1, 0:RD], lhsT=uT_sb[:], rhs=psall2[:, RD : 2 * RD]
                    )
                    t0 = pair * 2
                    b_sb = wpool.tile([P, 2 * RD], f32, tag="b")
                    vs_bc = (
                        vs_g[:, t0 * D : (t0 + 2) * D]
                        .rearrange("p (t o d) -> p t o d", t=2, o=1)
                        .to_broadcast([P, 2, R, D])
                    )
                    nc.vector.tensor_mul(
                        b_sb[:].rearrange("p (t r d) -> p t r d", t=2, r=R),
                        z_ps[:, :, 0:RD].rearrange("p t (r d) -> p t r d", r=R),
                        vs_bc,
                    )
                    nc.vector.tensor_reduce(
                        out=scores[:, t0 * R : (t0 + 2) * R],
                        in_=b_sb[:].rearrange("p (t r d) -> p t r d", t=2, r=R),
                        axis=mybir.AxisListType.X,
                        op=mybir.AluOpType.add,
                    )

                e_t = wpool.tile([P, TILES_PER_BLK * R], f32, tag="e")
                nc.scalar.activation(
                    e_t[:, : nt * R],
                    scores[:, : nt * R],
                    mybir.ActivationFunctionType.Exp,
                )
                den = wpool.tile([P, TILES_PER_BLK], f32, tag="den")
                nc.vector.tensor_reduce(
                    out=den[:, :nt],
                    in_=e_t[:, : nt * R].rearrange("p (t r) -> p t r", r=R),
                    axis=mybir.AxisListType.X,
                    op=mybir.AluOpType.add,
                )
                num_s = wpool.tile([P, TILES_PER_BLK * R], f32, tag="nums")
                vals_bc = (
                    vals_t[:]
                    .rearrange("p (o r) -> p o r", o=1)
                    .to_broadcast([P, nt, R])
                )
                nc.vector.tensor_mul(
                    num_s[:, : nt * R].rearrange("p (t r) -> p t r", r=R),
                    e_t[:, : nt * R].rearrange("p (t r) -> p t r", r=R),
                    vals_bc,
                )
                num = wpool.tile([P, TILES_PER_BLK], f32, tag="num")
                nc.vector.tensor_reduce(
                    out=num[:, :nt],
                    in_=num_s[:, : nt * R].rearrange("p (t r) -> p t r", r=R),
                    axis=mybir.AxisListType.X,
                    op=mybir.AluOpType.add,
                )
                rden = wpool.tile([P, TILES_PER_BLK], f32, tag="rden")
                nc.vector.reciprocal(rden[:, :nt], den[:, :nt])
                rat = wpool.tile([P, TILES_PER_BLK], f32, tag="rat")
                nc.vector.tensor_mul(rat[:, :nt], num[:, :nt], rden[:, :nt])
                nc.sync.dma_start(
                    out_d[:, b * TILES_PER_BLK : b * TILES_PER_BLK + nt], rat[:, :nt]
                )
    nc.compile()
    return nc


def _prepare(ufeat, ifeat, Ps, src, dst):
    perm = np.argsort(dst, kind="stable")
    psall = np.ascontiguousarray(Ps.transpose(1, 0, 2).reshape(D, R * D))
    psall2 = np.zeros((P, 2 * R * D), np.float32)
    psall2[:D, : R * D] = psall
    psall2[D:, R * D :] = psall
    vals = np.tile(np.arange(1.0, 6.0, dtype=np.float32), (P, 1)).astype(np.float32)

    in_maps, metas = [], []
    for c in range(N_CORES):
        eids = perm[c * E_CORE : (c + 1) * E_CORE]
        eids = eids[np.argsort(src[eids], kind="stable")]
        s = src[eids].astype(np.int64)
        d = dst[eids].astype(np.int64)
        d_lo = int(d.min())
        width = int(d.max()) - d_lo + 1
        assert width <= IF_ROWS, width
        if_c = np.zeros((IF_ROWS, D), np.float32)
        if_c[:width] = ifeat[d_lo : d_lo + width]

        idxu16 = np.zeros(PAD_E, np.int16)
        idxv16 = np.zeros(PAD_E, np.int16)
        for b in range(N_BLK):
            lo, hi = b * BLK, min((b + 1) * BLK, E_CORE)
            su = s[lo:hi] - _WIN_OFFS[b]
            assert su.min() >= 0 and su.max() < WIN, (b, su.min(), su.max())
            idxu16[b * BLK : b * BLK + (hi - lo)] = su.astype(np.int16)
            idxv16[b * BLK : b * BLK + (hi - lo)] = (d[lo:hi] - d_lo).astype(np.int16)

        def wrap(a):
            cols = np.concatenate(
                [
                    a[g * GBLK : (g + 1) * GBLK].reshape(GBLK // 16, 16).T
                    for g in range(PAD_E // GBLK)
                ],
                axis=1,
            )
            return np.tile(cols, (8, 1)).astype(np.int16)

        in_maps.append(
            {
                "ufeat": np.ascontiguousarray(ufeat, np.float32),
                "ifeat_c": if_c,
                "psall2": psall2,
                "idxu": wrap(idxu16),
                "idxv": wrap(idxv16),
                "vals": vals,
            }
        )
        metas.append(eids)
    return in_maps, metas


def _install_profile_hook():
    """Make antenv.axon_hooks available so run_bass_kernel_spmd(trace=True)
    can capture NTFF profiles through the axon .so (used by test.py only)."""
    import types

    try:
        from antenv.axon_hooks import get_axon_ntff_profile_hook  # noqa: F401

        return
    except ImportError:
        pass
    import antenv
    from trn_agent_boot.trn_boot import _ntff_profile_via_ctypes

    hook = _ntff_profile_via_ctypes("/opt/axon/libaxon_pjrt.so")
    mod = types.ModuleType("antenv.axon_hooks")
    mod._hook = hook
    mod.get_axon_ntff_profile_hook = lambda: mod._hook
    mod.set_axon_ntff_profile_hook = lambda h: setattr(mod, "_hook", h)
    sys.modules["antenv.axon_hooks"] = mod
    antenv.axon_hooks = mod


def kernel(ufeat, ifeat, Ps, src, dst):
    from concourse.bass_utils import run_bass_kernel_spmd

    ufeat = np.asarray(ufeat, np.float32)
    ifeat = np.asarray(ifeat, np.float32)
    Ps = np.asarray(Ps, np.float32)
    src = np.asarray(src, np.int32)
    dst = np.asarray(dst, np.int32)

    if "nc" not in _NC_CACHE:
        _NC_CACHE["nc"] = _build_kernel()
    nc = _NC_CACHE["nc"]
    in_maps, metas = _prepare(ufeat, ifeat, Ps, src, dst)
    res = run_bass_kernel_spmd(nc, in_maps, core_ids=list(range(N_CORES)))
    out = np.zeros(E, np.float32)
    for c in range(N_CORES):
        o = res.results[c]["out"].reshape(P, N_BLK, TILES_PER_BLK)
        flat = o.transpose(1, 2, 0).reshape(-1)
        out[metas[c]] = flat[:E_CORE]
    return out



# revision 10
# speedup vs baseline: 3.2746x; 3.2746x over previous
"""TRN2 Bass kernel for nn_BiDecoder (GNN edge rating decoder), 8 NeuronCores.

ratings[e] = sum_r softmax_r(ufeat[src[e]] @ Ps[r] @ ifeat[dst[e]]) * (r+1)

v2 design:
  - Edges dst-sorted into 8 contiguous shards (each core owns an item band).
  - Within a core, items are covered by NW aligned 128-item windows; edges are
    grouped by (window w, src-quarter q) and sorted by dst inside each group.
    Each group gets G slots (G multiple of 256); trailing pads idx=-1.
  - Item side needs NO gather: Y[j, (r,d)] = sum_f Ps[r,d,f] ifeat[j,f] is
    precomputed per window on PE (fp16); per tile a one-hot SelT (built with
    tensor_mask_reduce run-intervals, since dst-sorted edges give each item a
    contiguous slot run) expands Y rows to edges via one 320-col matmul.
  - User side: dma_gather of ufeat rows, 4 src-quarters (int16 idx < 25000)
    issued round-robin on 4 SWDGE queues so descriptor generation runs on all
    Q7 cpu pairs in parallel.
  - scores = sum_d us*yv via fp16 DVE mul (2x) + binary-tree adds (2x),
    softmax (no max-sub; scores bounded) -> ratings.
"""
import sys

sys.path.insert(0, "/opt/trn_rl_repo")
import numpy as np

P = 128
D = 64
R = 5
RD = R * D
N_USERS, N_ITEMS, E = 100000, 50000, 1000000
N_CORES = 8
E_CORE = E // N_CORES
NQ = 4
QS = N_USERS // NQ  # 25000

_NC_CACHE = {}


def _plan(src, dst):
    """Shard + group edges; derive (NW, G) shared across cores."""
    perm = np.argsort(dst, kind="stable")
    shards = perm.reshape(N_CORES, E_CORE)
    cores = []
    maxw = 0
    maxcnt = 0
    for c in range(N_CORES):
        eid = shards[c]
        d = dst[eid].astype(np.int64)
        s = src[eid].astype(np.int64)
        d_lo = int(d[0])
        nw = -(-(int(d[-1]) - d_lo + 1) // P)
        maxw = max(maxw, nw)
        w = (d - d_lo) >> 7
        q = s // QS
        order = np.lexsort((d, q, w))
        eid, d, s, w, q = eid[order], d[order], s[order], w[order], q[order]
        g = w * NQ + q
        cnt = np.bincount(g, minlength=nw * NQ)
        maxcnt = max(maxcnt, int(cnt.max()))
        cores.append((eid, d, s, w, q, g, d_lo, nw, cnt))
    NW = maxw
    G = max(256, -(-maxcnt // 256) * 256)
    return cores, NW, G


def _prepare(ufeat, ifeat, Ps, src, dst):
    ufeat = np.asarray(ufeat, np.float32)
    ifeat = np.asarray(ifeat, np.float32)
    Ps = np.asarray(Ps, np.float32)
    src = np.asarray(src)
    dst = np.asarray(dst)
    cores, NW, G = _plan(src, dst)
    _NC_CACHE["params"] = (NW, G)
    S16 = G // 16
    NG = NW * NQ
    p2 = np.ascontiguousarray(Ps.transpose(2, 0, 1).reshape(D, RD)).astype(np.float16)
    vals = np.tile(np.arange(1.0, 6.0, dtype=np.float32), (P, 1))
    uf = np.ascontiguousarray(ufeat)
    in_maps, metas = [], []
    for (eid, d, s, w, q, g, d_lo, nw, cnt) in cores:
        nslot = NG * G
        starts = np.zeros(NG + 1, np.int64)
        starts[1 : len(cnt) + 1] = np.cumsum(cnt)
        pos = np.arange(E_CORE) - starts[g]
        slot = g * G + pos
        idxu = np.zeros(nslot, np.int16)
        idxu[slot] = (s - q * QS).astype(np.int16)
        slot2eid = np.full(nslot, -1, np.int64)
        slot2eid[slot] = eid
        dl = d - d_lo - (w << 7)  # 0..127 within window
        # per-slot dst-local row id; pads get 128 (matches no iota row -> Sel 0)
        dstloc = np.full(nslot, 128, np.float16)
        dstloc[slot] = dl.astype(np.float16)
        wrapped = (
            idxu.reshape(NG, S16, 16).transpose(2, 0, 1).reshape(16, NG * S16)
        )  # [16, (group-major, col-minor)]; element i of group = [i%16, i//16]
        wrapped = np.tile(wrapped, (8, 1)).astype(np.int16)
        band = np.zeros((NW * P, D), np.float32)
        navail = min(NW * P, N_ITEMS - d_lo)
        band[:navail] = ifeat[d_lo : d_lo + navail]
        ifT = np.ascontiguousarray(band.T).astype(np.float16)
        in_maps.append(
            {
                "ufeat": uf,
                "ifT": ifT,
                "p2": p2,
                "idxu": wrapped,
                "dstloc": dstloc.reshape(NW, NQ * G),
                "iota": np.arange(P, dtype=np.float32).reshape(P, 1),
                "vals": vals,
            }
        )
        metas.append(slot2eid)
    return in_maps, metas


def _emit(nc, tc, aps, NW, G):
    import concourse.mybir as mybir
    from concourse import library_config

    f32, f16, i16 = mybir.dt.float32, mybir.dt.float16, mybir.dt.int16
    AF = mybir.ActivationFunctionType
    ALU = mybir.AluOpType
    AX = mybir.AxisListType
    TPW = NQ * G // P
    NG = NW * NQ
    S16 = G // 16
    GT = G // P  # tiles per group
    ufeat_d, ifT_d, p2_d, idxu_d, dstloc_d, iota_d, vals_d, out_d = aps

    nc.gpsimd.load_library(library_config.mlp)
    with tc.tile_pool(name="const", bufs=1) as cpool:
        p2_sb = cpool.tile([D, RD], f16)
        nc.sync.dma_start(p2_sb[:], p2_d[:])
        ifT_sb = cpool.tile([D, NW * P], f16)
        nc.sync.dma_start(ifT_sb[:], ifT_d[:])
        idx_sb = cpool.tile([P, NG * S16], i16)
        nc.sync.dma_start(idx_sb[:], idxu_d[:])
        iota_sb = cpool.tile([P, 1], f32)
        nc.sync.dma_start(iota_sb[:], iota_d[:])
        vals_sb = cpool.tile([P, R], f32)
        nc.sync.dma_start(vals_sb[:], vals_d[:])
        ysb = cpool.tile([P, NW, RD], f16)
        outbuf = cpool.tile([P, NW * TPW], f32)

        with tc.tile_pool(name="ypsum", bufs=2, space="PSUM") as ypool:
            for w in range(NW):
                y_ps = ypool.tile([P, 512], f32, tag="y")
                nc.tensor.matmul(
                    y_ps[:, 0:RD], lhsT=ifT_sb[:, w * P : (w + 1) * P], rhs=p2_sb[:]
                )
                nc.scalar.activation(ysb[:, w, :], y_ps[:, 0:RD], AF.Copy)

        with (
            tc.tile_pool(name="gather", bufs=3) as gpool,
            tc.tile_pool(name="work", bufs=2) as wpool,
            tc.tile_pool(name="psum_yv", bufs=2, space="PSUM") as zpool,
        ):
            for w in range(NW):
                us_f = gpool.tile([P, TPW, D], f32, tag="usf")
                for q in range(NQ):
                    gi = w * NQ + q
                    nc.gpsimd.dma_gather(
                        out_ap=us_f[:, q * GT : (q + 1) * GT, :],
                        in_ap=ufeat_d[q * QS : (q + 1) * QS, :],
                        idxs_ap=idx_sb[:, gi * S16 : (gi + 1) * S16],
                        num_idxs=G,
                        num_idxs_reg=G,
                        elem_size=D,
                        queue_num=q,
                    )
                us_h = wpool.tile([P, TPW * D], f16, tag="ush")
                nc.scalar.activation(
                    us_h[:], us_f[:].rearrange("p t d -> p (t d)"), AF.Copy
                )
                dl_sb = gpool.tile([P, NQ * G], f16, tag="dl")
                nc.sync.dma_start(
                    dl_sb[:], dstloc_d[w : w + 1, :].to_broadcast([P, NQ * G])
                )
                selt = wpool.tile([P, NQ * G], f16, tag="selt")
                nc.vector.tensor_scalar(
                    out=selt[:],
                    in0=dl_sb[:],
                    scalar1=iota_sb[:],
                    scalar2=None,
                    op0=ALU.is_equal,
                )
                scorew = wpool.tile([P, TPW, R], f32, tag="scw")
                for half in range(TPW // 8):
                    b8 = wpool.tile([P, 8, R, D], f16, tag="b8")
                    for sub in range(2):
                        yv_ps = zpool.tile([P, 4, 512], f32, tag="yv")
                        for i in range(4):
                            t = half * 8 + sub * 4 + i
                            nc.tensor.matmul(
                                yv_ps[:, i, 0:RD],
                                lhsT=selt[:, t * P : (t + 1) * P],
                                rhs=ysb[:, w, :],
                            )
                        yv_h = wpool.tile([P, 4, RD], f16, tag="yvh")
                        nc.scalar.activation(yv_h[:], yv_ps[:, :, 0:RD], AF.Copy)
                        t0 = half * 8 + sub * 4
                        nc.vector.tensor_mul(
                            b8[:, sub * 4 : (sub + 1) * 4, :, :],
                            us_h[:, t0 * D : (t0 + 4) * D]
                            .rearrange("p (t o d) -> p t o d", o=1, d=D)
                            .to_broadcast([P, 4, R, D]),
                            yv_h[:].rearrange("p t (r d) -> p t r d", r=R),
                        )
                    t32 = wpool.tile([P, 8, R, 32], f16, tag="t32")
                    nc.vector.tensor_add(t32[:], b8[:, :, :, 0:32], b8[:, :, :, 32:64])
                    t16 = wpool.tile([P, 8, R, 16], f16, tag="t16")
                    nc.vector.tensor_add(t16[:], t32[:, :, :, 0:16], t32[:, :, :, 16:32])
                    t8 = wpool.tile([P, 8, R, 8], f16, tag="t8")
                    nc.vector.tensor_add(t8[:], t16[:, :, :, 0:8], t16[:, :, :, 8:16])
                    t4 = wpool.tile([P, 8, R, 4], f32, tag="t4")
                    nc.vector.tensor_add(t4[:], t8[:, :, :, 0:4], t8[:, :, :, 4:8])
                    nc.vector.tensor_reduce(
                        out=scorew[:, half * 8 : (half + 1) * 8, :],
                        in_=t4[:],
                        axis=AX.X,
                        op=ALU.add,
                    )
                e_t = wpool.tile([P, TPW * R], f32, tag="et")
                nc.scalar.activation(
                    e_t[:], scorew[:].rearrange("p t r -> p (t r)"), AF.Exp
                )
                den = wpool.tile([P, TPW], f32, tag="den")
                nc.vector.tensor_reduce(
                    out=den[:],
                    in_=e_t[:].rearrange("p (t r) -> p t r", r=R),
                    axis=AX.X,
                    op=ALU.add,
                )
                nums = wpool.tile([P, TPW * R], f32, tag="nums")
                vals_bc = (
                    vals_sb[:]
                    .rearrange("p (o r) -> p o r", o=1)
                    .to_broadcast([P, TPW, R])
                )
                nc.vector.tensor_mul(
                    nums[:].rearrange("p (t r) -> p t r", r=R),
                    e_t[:].rearrange("p (t r) -> p t r", r=R),
                    vals_bc,
                )
                num = wpool.tile([P, TPW], f32, tag="num")
                nc.vector.tensor_reduce(
                    out=num[:],
                    in_=nums[:].rearrange("p (t r) -> p t r", r=R),
                    axis=AX.X,
                    op=ALU.add,
                )
                rden = wpool.tile([P, TPW], f32, tag="rden")
                nc.vector.reciprocal(rden[:], den[:])
                nc.vector.tensor_mul(
                    outbuf[:, w * TPW : (w + 1) * TPW], num[:], rden[:]
                )
            nc.sync.dma_start(out_d[:], outbuf[:])


def _build(NW, G):
    import concourse.bacc as bacc
    import concourse.mybir as mybir
    import concourse.tile as tile

    nc = bacc.Bacc(None, target_bir_lowering=False, num_swdge_queues=NQ)
    f32, f16, i16 = mybir.dt.float32, mybir.dt.float16, mybir.dt.int16
    TPW = NQ * G // P
    NG = NW * NQ
    ufeat_d = nc.dram_tensor("ufeat", [N_USERS, D], f32, kind="ExternalInput")
    ifT_d = nc.dram_tensor("ifT", [D, NW * P], f16, kind="ExternalInput")
    p2_d = nc.dram_tensor("p2", [D, RD], f16, kind="ExternalInput")
    idxu_d = nc.dram_tensor("idxu", [P, NG * (G // 16)], i16, kind="ExternalInput")
    dstloc_d = nc.dram_tensor("dstloc", [NW, NQ * G], f16, kind="ExternalInput")
    iota_d = nc.dram_tensor("iota", [P, 1], f32, kind="ExternalInput")
    vals_d = nc.dram_tensor("vals", [P, R], f32, kind="ExternalInput")
    out_d = nc.dram_tensor("out", [P, NW * TPW], f32, kind="ExternalOutput")

    with tile.TileContext(nc) as tc:
        _emit(
            nc,
            tc,
            (ufeat_d, ifT_d, p2_d, idxu_d, dstloc_d, iota_d, vals_d, out_d),
            NW,
            G,
        )
    nc.compile()
    return nc


def _install_profile_hook():
    """Make antenv.axon_hooks available so run_bass_kernel_spmd(trace=True)
    can capture NTFF profiles through the axon .so (used by test.py only)."""
    import types

    try:
        from antenv.axon_hooks import get_axon_ntff_profile_hook  # noqa: F401

        return
    except ImportError:
        pass
    import antenv
    from trn_agent_boot.trn_boot import _ntff_profile_via_ctypes

    hook = _ntff_profile_via_ctypes("/opt/axon/libaxon_pjrt.so")
    mod = types.ModuleType("antenv.axon_hooks")
    mod._hook = hook
    mod.get_axon_ntff_profile_hook = lambda: mod._hook
    mod.set_axon_ntff_profile_hook = lambda h: setattr(mod, "_hook", h)
    sys.modules["antenv.axon_hooks"] = mod
    antenv.axon_hooks = mod


def kernel(ufeat, ifeat, Ps, src, dst):
    from concourse.bass_utils import run_bass_kernel_spmd

    ufeat = np.asarray(ufeat, np.float32)
    ifeat = np.asarray(ifeat, np.float32)
    Ps = np.asarray(Ps, np.float32)
    src = np.asarray(src, np.int32)
    dst = np.asarray(dst, np.int32)

    in_maps, metas = _prepare(ufeat, ifeat, Ps, src, dst)
    NW, G = _NC_CACHE["params"]
    key = ("nc", NW, G)
    if key not in _NC_CACHE:
        _NC_CACHE[key] = _build(NW, G)
        _NC_CACHE["nc"] = _NC_CACHE[key]
    nc = _NC_CACHE[key]
    res = run_bass_kernel_spmd(nc, in_maps, core_ids=list(range(N_CORES)))
    out = np.zeros(E, np.float32)
    for c in range(N_CORES):
        o = res.results[c]["out"]  # [P, NW*TPW]
        flat = o.T.reshape(-1)  # slot-ordered
        s2e = metas[c]
        valid = s2e >= 0
        out[s2e[valid]] = flat[valid]
    return out


# revision 11
# speedup vs baseline: 3.7875x; 1.1566x over previous
"""TRN2 Bass kernel for nn_BiDecoder (GNN edge rating decoder), 8 NeuronCores.

ratings[e] = sum_r softmax_r(ufeat[src[e]] @ Ps[r] @ ifeat[dst[e]]) * (r+1)

v2 design:
  - Edges dst-sorted into 8 contiguous shards (each core owns an item band).
  - Within a core, items are covered by NW aligned 128-item windows; edges are
    grouped by (window w, src-quarter q) and sorted by dst inside each group.
    Each group gets G slots (G multiple of 256); trailing pads idx=-1.
  - Item side needs NO gather: Y[j, (r,d)] = sum_f Ps[r,d,f] ifeat[j,f] is
    precomputed per window on PE (fp16); per tile a one-hot SelT (built with
    tensor_mask_reduce run-intervals, since dst-sorted edges give each item a
    contiguous slot run) expands Y rows to edges via one 320-col matmul.
  - User side: dma_gather of ufeat rows, 4 src-quarters (int16 idx < 25000)
    issued round-robin on 4 SWDGE queues so descriptor generation runs on all
    Q7 cpu pairs in parallel.
  - scores = sum_d us*yv via fp16 DVE mul (2x) + binary-tree adds (2x),
    softmax (no max-sub; scores bounded) -> ratings.
"""
import sys

sys.path.insert(0, "/opt/trn_rl_repo")
import numpy as np

P = 128
D = 64
R = 5
RD = R * D
N_USERS, N_ITEMS, E = 100000, 50000, 1000000
N_CORES = 8
E_CORE = E // N_CORES
NQ = 4
QS = N_USERS // NQ  # 25000

_NC_CACHE = {}


def _plan(src, dst):
    """Shard + group edges; derive (NW, G) shared across cores."""
    perm = np.argsort(dst, kind="stable")
    shards = perm.reshape(N_CORES, E_CORE)
    cores = []
    maxw = 0
    maxcnt = 0
    for c in range(N_CORES):
        eid = shards[c]
        d = dst[eid].astype(np.int64)
        s = src[eid].astype(np.int64)
        d_lo = int(d[0])
        nw = -(-(int(d[-1]) - d_lo + 1) // P)
        maxw = max(maxw, nw)
        w = (d - d_lo) >> 7
        q = s // QS
        order = np.lexsort((d, q, w))
        eid, d, s, w, q = eid[order], d[order], s[order], w[order], q[order]
        g = w * NQ + q
        cnt = np.bincount(g, minlength=nw * NQ)
        maxcnt = max(maxcnt, int(cnt.max()))
        cores.append((eid, d, s, w, q, g, d_lo, nw, cnt))
    NW = maxw
    G = max(256, -(-maxcnt // 256) * 256)
    return cores, NW, G


def _prepare(ufeat, ifeat, Ps, src, dst):
    ufeat = np.asarray(ufeat, np.float32)
    ifeat = np.asarray(ifeat, np.float32)
    Ps = np.asarray(Ps, np.float32)
    src = np.asarray(src)
    dst = np.asarray(dst)
    cores, NW, G = _plan(src, dst)
    _NC_CACHE["params"] = (NW, G)
    S16 = G // 16
    NG = NW * NQ
    p2 = np.ascontiguousarray(Ps.transpose(2, 0, 1).reshape(D, RD)).astype(np.float16)
    vals = np.tile(np.arange(1.0, 6.0, dtype=np.float32), (P, 1))
    uf = np.zeros((N_USERS, 2 * D), np.float16)
    uf[:, :D] = ufeat.astype(np.float16)
    iotar = np.tile(np.arange(P, dtype=np.float16).reshape(P, 1), (1, NQ * G))
    in_maps, metas = [], []
    for (eid, d, s, w, q, g, d_lo, nw, cnt) in cores:
        nslot = NG * G
        starts = np.zeros(NG + 1, np.int64)
        starts[1 : len(cnt) + 1] = np.cumsum(cnt)
        pos = np.arange(E_CORE) - starts[g]
        slot = g * G + pos
        idxu = np.zeros(nslot, np.int16)
        idxu[slot] = (s - q * QS).astype(np.int16)
        slot2eid = np.full(nslot, -1, np.int64)
        slot2eid[slot] = eid
        dl = d - d_lo - (w << 7)  # 0..127 within window
        # per-slot dst-local row id; pads get 128 (matches no iota row -> Sel 0)
        dstloc = np.full(nslot, 128, np.float16)
        dstloc[slot] = dl.astype(np.float16)
        wrapped = (
            idxu.reshape(NG, S16, 16).transpose(2, 0, 1).reshape(16, NG * S16)
        )  # [16, (group-major, col-minor)]; element i of group = [i%16, i//16]
        wrapped = np.tile(wrapped, (8, 1)).astype(np.int16)
        band = np.zeros((NW * P, D), np.float32)
        navail = min(NW * P, N_ITEMS - d_lo)
        band[:navail] = ifeat[d_lo : d_lo + navail]
        ifT = np.ascontiguousarray(band.T).astype(np.float16)
        in_maps.append(
            {
                "ufeat": uf,
                "ifT": ifT,
                "p2": p2,
                "idxu": wrapped,
                "dstloc": dstloc.reshape(NW, NQ * G),
                "iotar": iotar,
                "vals": vals,
            }
        )
        metas.append(slot2eid)
    return in_maps, metas


def _emit(nc, tc, aps, NW, G):
    import concourse.mybir as mybir
    from concourse import library_config

    f32, f16, i16 = mybir.dt.float32, mybir.dt.float16, mybir.dt.int16
    AF = mybir.ActivationFunctionType
    ALU = mybir.AluOpType
    AX = mybir.AxisListType
    TPW = NQ * G // P
    NG = NW * NQ
    S16 = G // 16
    GT = G // P  # tiles per group
    ufeat_d, ifT_d, p2_d, idxu_d, dstloc_d, iotar_d, vals_d, out_d = aps

    nc.gpsimd.load_library(library_config.mlp)
    with tc.tile_pool(name="const", bufs=1) as cpool:
        p2_sb = cpool.tile([D, RD], f16)
        nc.sync.dma_start(p2_sb[:], p2_d[:])
        ifT_sb = cpool.tile([D, NW * P], f16)
        nc.sync.dma_start(ifT_sb[:], ifT_d[:])
        idx_sb = cpool.tile([P, NG * S16], i16)
        nc.sync.dma_start(idx_sb[:], idxu_d[:])
        iotar_sb = cpool.tile([P, NQ * G], f16)
        nc.sync.dma_start(iotar_sb[:], iotar_d[:])
        vals_sb = cpool.tile([P, R], f32)
        nc.sync.dma_start(vals_sb[:], vals_d[:])
        ysb = cpool.tile([P, NW, RD], f16)
        outbuf = cpool.tile([P, NW * TPW], f32)

        with tc.tile_pool(name="ypsum", bufs=2, space="PSUM") as ypool:
            for w in range(NW):
                y_ps = ypool.tile([P, 512], f32, tag="y")
                nc.tensor.matmul(
                    y_ps[:, 0:RD], lhsT=ifT_sb[:, w * P : (w + 1) * P], rhs=p2_sb[:]
                )
                nc.scalar.activation(ysb[:, w, :], y_ps[:, 0:RD], AF.Copy)

        with (
            tc.tile_pool(name="gather", bufs=3) as gpool,
            tc.tile_pool(name="work", bufs=2) as wpool,
            tc.tile_pool(name="psum_yv", bufs=2, space="PSUM") as zpool,
        ):
            for w in range(NW):
                us_f = gpool.tile([P, TPW, 2 * D], f16, tag="usf")
                for q in range(NQ):
                    gi = w * NQ + q
                    nc.gpsimd.dma_gather(
                        out_ap=us_f[:, q * GT : (q + 1) * GT, :],
                        in_ap=ufeat_d[q * QS : (q + 1) * QS, :],
                        idxs_ap=idx_sb[:, gi * S16 : (gi + 1) * S16],
                        num_idxs=G,
                        num_idxs_reg=G,
                        elem_size=2 * D,
                        queue_num=q,
                    )
                dl_sb = gpool.tile([P, NQ * G], f16, tag="dl")
                nc.sync.dma_start(
                    dl_sb[:], dstloc_d[w : w + 1, :].to_broadcast([P, NQ * G])
                )
                selt = wpool.tile([P, NQ * G], f16, tag="selt")
                nc.vector.tensor_tensor(
                    selt[:], dl_sb[:], iotar_sb[:], op=ALU.is_equal
                )
                scorew = wpool.tile([P, TPW, R], f32, tag="scw")
                for half in range(TPW // 8):
                    b8 = wpool.tile([P, 8, R, D], f16, tag="b8")
                    for sub in range(2):
                        yv_ps = zpool.tile([P, 4, 512], f32, tag="yv")
                        for i in range(4):
                            t = half * 8 + sub * 4 + i
                            nc.tensor.matmul(
                                yv_ps[:, i, 0:RD],
                                lhsT=selt[:, t * P : (t + 1) * P],
                                rhs=ysb[:, w, :],
                            )
                        yv_h = wpool.tile([P, 4, RD], f16, tag="yvh")
                        nc.scalar.activation(yv_h[:], yv_ps[:, :, 0:RD], AF.Copy)
                        t0 = half * 8 + sub * 4
                        nc.vector.tensor_mul(
                            b8[:, sub * 4 : (sub + 1) * 4, :, :],
                            us_f[:, t0 : t0 + 4, 0:D]
                            .rearrange("p t (o d) -> p t o d", o=1)
                            .to_broadcast([P, 4, R, D]),
                            yv_h[:].rearrange("p t (r d) -> p t r d", r=R),
                        )
                    t32 = wpool.tile([P, 8, R, 32], f16, tag="t32")
                    nc.vector.tensor_add(t32[:], b8[:, :, :, 0:32], b8[:, :, :, 32:64])
                    t16 = wpool.tile([P, 8, R, 16], f16, tag="t16")
                    nc.vector.tensor_add(t16[:], t32[:, :, :, 0:16], t32[:, :, :, 16:32])
                    t8 = wpool.tile([P, 8, R, 8], f16, tag="t8")
                    nc.vector.tensor_add(t8[:], t16[:, :, :, 0:8], t16[:, :, :, 8:16])
                    t4 = wpool.tile([P, 8, R, 4], f32, tag="t4")
                    nc.vector.tensor_add(t4[:], t8[:, :, :, 0:4], t8[:, :, :, 4:8])
                    nc.vector.tensor_reduce(
                        out=scorew[:, half * 8 : (half + 1) * 8, :],
                        in_=t4[:],
                        axis=AX.X,
                        op=ALU.add,
                    )
                e_t = wpool.tile([P, TPW * R], f32, tag="et")
                nc.scalar.activation(
                    e_t[:], scorew[:].rearrange("p t r -> p (t r)"), AF.Exp
                )
                den = wpool.tile([P, TPW], f32, tag="den")
                nc.vector.tensor_reduce(
                    out=den[:],
                    in_=e_t[:].rearrange("p (t r) -> p t r", r=R),
                    axis=AX.X,
                    op=ALU.add,
                )
                nums = wpool.tile([P, TPW * R], f32, tag="nums")
                vals_bc = (
                    vals_sb[:]
                    .rearrange("p (o r) -> p o r", o=1)
                    .to_broadcast([P, TPW, R])
                )
                nc.vector.tensor_mul(
                    nums[:].rearrange("p (t r) -> p t r", r=R),
                    e_t[:].rearrange("p (t r) -> p t r", r=R),
                    vals_bc,
                )
                num = wpool.tile([P, TPW], f32, tag="num")
                nc.vector.tensor_reduce(
                    out=num[:],
                    in_=nums[:].rearrange("p (t r) -> p t r", r=R),
                    axis=AX.X,
                    op=ALU.add,
                )
                rden = wpool.tile([P, TPW], f32, tag="rden")
                nc.vector.reciprocal(rden[:], den[:])
                nc.vector.tensor_mul(
                    outbuf[:, w * TPW : (w + 1) * TPW], num[:], rden[:]
                )
            nc.sync.dma_start(out_d[:], outbuf[:])


def _build(NW, G):
    import concourse.bacc as bacc
    import concourse.mybir as mybir
    import concourse.tile as tile

    nc = bacc.Bacc(None, target_bir_lowering=False, num_swdge_queues=NQ)
    f32, f16, i16 = mybir.dt.float32, mybir.dt.float16, mybir.dt.int16
    TPW = NQ * G // P
    NG = NW * NQ
    ufeat_d = nc.dram_tensor("ufeat", [N_USERS, 2 * D], f16, kind="ExternalInput")
    ifT_d = nc.dram_tensor("ifT", [D, NW * P], f16, kind="ExternalInput")
    p2_d = nc.dram_tensor("p2", [D, RD], f16, kind="ExternalInput")
    idxu_d = nc.dram_tensor("idxu", [P, NG * (G // 16)], i16, kind="ExternalInput")
    dstloc_d = nc.dram_tensor("dstloc", [NW, NQ * G], f16, kind="ExternalInput")
    iotar_d = nc.dram_tensor("iotar", [P, NQ * G], f16, kind="ExternalInput")
    vals_d = nc.dram_tensor("vals", [P, R], f32, kind="ExternalInput")
    out_d = nc.dram_tensor("out", [P, NW * TPW], f32, kind="ExternalOutput")

    with tile.TileContext(nc) as tc:
        _emit(
            nc,
            tc,
            (ufeat_d, ifT_d, p2_d, idxu_d, dstloc_d, iotar_d, vals_d, out_d),
            NW,
            G,
        )
    nc.compile()
    return nc


def _install_profile_hook():
    """Make antenv.axon_hooks available so run_bass_kernel_spmd(trace=True)
    can capture NTFF profiles through the axon .so (used by test.py only)."""
    import types

    try:
        from antenv.axon_hooks import get_axon_ntff_profile_hook  # noqa: F401

        return
    except ImportError:
        pass
    import antenv
    from trn_agent_boot.trn_boot import _ntff_profile_via_ctypes

    hook = _ntff_profile_via_ctypes("/opt/axon/libaxon_pjrt.so")
    mod = types.ModuleType("antenv.axon_hooks")
    mod._hook = hook
    mod.get_axon_ntff_profile_hook = lambda: mod._hook
    mod.set_axon_ntff_profile_hook = lambda h: setattr(mod, "_hook", h)
    sys.modules["antenv.axon_hooks"] = mod
    antenv.axon_hooks = mod


def kernel(ufeat, ifeat, Ps, src, dst):
    from concourse.bass_utils import run_bass_kernel_spmd

    ufeat = np.asarray(ufeat, np.float32)
    ifeat = np.asarray(ifeat, np.float32)
    Ps = np.asarray(Ps, np.float32)
    src = np.asarray(src, np.int32)
    dst = np.asarray(dst, np.int32)

    in_maps, metas = _prepare(ufeat, ifeat, Ps, src, dst)
    NW, G = _NC_CACHE["params"]
    key = ("nc", NW, G)
    if key not in _NC_CACHE:
        _NC_CACHE[key] = _build(NW, G)
        _NC_CACHE["nc"] = _NC_CACHE[key]
    nc = _NC_CACHE[key]
    res = run_bass_kernel_spmd(nc, in_maps, core_ids=list(range(N_CORES)))
    out = np.zeros(E, np.float32)
    for c in range(N_CORES):
        o = res.results[c]["out"]  # [P, NW*TPW]
        flat = o.T.reshape(-1)  # slot-ordered
        s2e = metas[c]
        valid = s2e >= 0
        out[s2e[valid]] = flat[valid]
    return out


# revision 12
# speedup vs baseline: 4.4202x; 1.1670x over previous
"""TRN2 Bass kernel for nn_BiDecoder (GNN edge rating decoder), 8 NeuronCores.

ratings[e] = sum_r softmax_r(ufeat[src[e]] @ Ps[r] @ ifeat[dst[e]]) * (r+1)

v2 design:
  - Edges dst-sorted into 8 contiguous shards (each core owns an item band).
  - Within a core, items are covered by NW aligned 128-item windows; edges are
    grouped by (window w, src-quarter q) and sorted by dst inside each group.
    Each group gets G slots (G multiple of 256); trailing pads idx=-1.
  - Item side needs NO gather: Y[j, (r,d)] = sum_f Ps[r,d,f] ifeat[j,f] is
    precomputed per window on PE (fp16); per tile a one-hot SelT (built with
    tensor_mask_reduce run-intervals, since dst-sorted edges give each item a
    contiguous slot run) expands Y rows to edges via one 320-col matmul.
  - User side: dma_gather of ufeat rows, 4 src-quarters (int16 idx < 25000)
    issued round-robin on 4 SWDGE queues so descriptor generation runs on all
    Q7 cpu pairs in parallel.
  - scores = sum_d us*yv via fp16 DVE mul (2x) + binary-tree adds (2x),
    softmax (no max-sub; scores bounded) -> ratings.
"""
import sys

sys.path.insert(0, "/opt/trn_rl_repo")
import numpy as np

P = 128
D = 64
R = 5
RD = R * D
N_USERS, N_ITEMS, E = 100000, 50000, 1000000
N_CORES = 8
E_CORE = E // N_CORES
NQ = 4
QS = N_USERS // NQ  # 25000

_NC_CACHE = {}


def _plan(src, dst):
    """Shard + group edges; derive (NW, G) shared across cores."""
    perm = np.argsort(dst, kind="stable")
    shards = perm.reshape(N_CORES, E_CORE)
    cores = []
    maxw = 0
    maxcnt = 0
    for c in range(N_CORES):
        eid = shards[c]
        d = dst[eid].astype(np.int64)
        s = src[eid].astype(np.int64)
        d_lo = int(d[0])
        nw = -(-(int(d[-1]) - d_lo + 1) // P)
        maxw = max(maxw, nw)
        w = (d - d_lo) >> 7
        q = s // QS
        order = np.lexsort((d, q, w))
        eid, d, s, w, q = eid[order], d[order], s[order], w[order], q[order]
        g = w * NQ + q
        cnt = np.bincount(g, minlength=nw * NQ)
        maxcnt = max(maxcnt, int(cnt.max()))
        cores.append((eid, d, s, w, q, g, d_lo, nw, cnt))
    NW = maxw
    G = max(256, -(-maxcnt // 256) * 256)
    return cores, NW, G


def _prepare(ufeat, ifeat, Ps, src, dst):
    ufeat = np.asarray(ufeat, np.float32)
    ifeat = np.asarray(ifeat, np.float32)
    Ps = np.asarray(Ps, np.float32)
    src = np.asarray(src)
    dst = np.asarray(dst)
    cores, NW, G = _plan(src, dst)
    _NC_CACHE["params"] = (NW, G)
    S16 = G // 16
    NG = NW * NQ
    p2 = np.ascontiguousarray(Ps.transpose(2, 0, 1).reshape(D, RD)).astype(np.float16)
    vals = np.tile(np.arange(1.0, 6.0, dtype=np.float32), (P, 1))
    uf = np.zeros((N_USERS, 2 * D), np.float16)
    uf[:, :D] = ufeat.astype(np.float16)
    in_maps, metas = [], []
    for (eid, d, s, w, q, g, d_lo, nw, cnt) in cores:
        nslot = NG * G
        starts = np.zeros(NG + 1, np.int64)
        starts[1 : len(cnt) + 1] = np.cumsum(cnt)
        pos = np.arange(E_CORE) - starts[g]
        slot = g * G + pos
        idxu = np.zeros(nslot, np.int16)
        idxu[slot] = (s - q * QS).astype(np.int16)
        slot2eid = np.full(nslot, -1, np.int64)
        slot2eid[slot] = eid
        dl = d - d_lo - (w << 7)  # 0..127 within window
        # host-built one-hot SelT: [row j, slot] = 1 iff slot's dst-local == j
        selt_h = np.zeros((P, nslot), np.float16)
        selt_h[dl, slot] = 1.0
        wrapped = (
            idxu.reshape(NG, S16, 16).transpose(2, 0, 1).reshape(16, NG * S16)
        )  # [16, (group-major, col-minor)]; element i of group = [i%16, i//16]
        wrapped = np.tile(wrapped, (8, 1)).astype(np.int16)
        band = np.zeros((NW * P, D), np.float32)
        navail = min(NW * P, N_ITEMS - d_lo)
        band[:navail] = ifeat[d_lo : d_lo + navail]
        ifT = np.ascontiguousarray(band.T).astype(np.float16)
        in_maps.append(
            {
                "ufeat": uf,
                "ifT": ifT,
                "p2": p2,
                "idxu": wrapped,
                "selt": np.ascontiguousarray(
                    selt_h.reshape(P, NW, NQ * G).transpose(1, 0, 2).reshape(
                        NW * P, NQ * G
                    )
                ),
                "vals": vals,
            }
        )
        metas.append(slot2eid)
    return in_maps, metas


def _emit(nc, tc, aps, NW, G):
    import concourse.mybir as mybir
    from concourse import library_config

    f32, f16, i16 = mybir.dt.float32, mybir.dt.float16, mybir.dt.int16
    AF = mybir.ActivationFunctionType
    ALU = mybir.AluOpType
    AX = mybir.AxisListType
    TPW = NQ * G // P
    NG = NW * NQ
    S16 = G // 16
    GT = G // P  # tiles per group
    ufeat_d, ifT_d, p2_d, idxu_d, selt_d, vals_d, out_d = aps

    nc.gpsimd.load_library(library_config.mlp)
    with tc.tile_pool(name="const", bufs=1) as cpool:
        p2_sb = cpool.tile([D, RD], f16)
        nc.sync.dma_start(p2_sb[:], p2_d[:])
        ifT_sb = cpool.tile([D, NW * P], f16)
        nc.sync.dma_start(ifT_sb[:], ifT_d[:])
        idx_sb = cpool.tile([P, NG * S16], i16)
        nc.sync.dma_start(idx_sb[:], idxu_d[:])
        vals_sb = cpool.tile([P, R], f32)
        nc.sync.dma_start(vals_sb[:], vals_d[:])
        ysb = cpool.tile([P, NW, RD], f16)
        outbuf = cpool.tile([P, NW * TPW], f32)

        with tc.tile_pool(name="ypsum", bufs=2, space="PSUM") as ypool:
            for w in range(NW):
                y_ps = ypool.tile([P, 512], f32, tag="y")
                nc.tensor.matmul(
                    y_ps[:, 0:RD], lhsT=ifT_sb[:, w * P : (w + 1) * P], rhs=p2_sb[:]
                )
                nc.scalar.activation(ysb[:, w, :], y_ps[:, 0:RD], AF.Copy)

        with (
            tc.tile_pool(name="gather", bufs=3) as gpool,
            tc.tile_pool(name="work", bufs=2) as wpool,
            tc.tile_pool(name="psum_yv", bufs=2, space="PSUM") as zpool,
        ):
            for w in range(NW):
                us_f = gpool.tile([P, TPW, 2 * D], f16, tag="usf")
                for q in range(NQ):
                    gi = w * NQ + q
                    nc.gpsimd.dma_gather(
                        out_ap=us_f[:, q * GT : (q + 1) * GT, :],
                        in_ap=ufeat_d[q * QS : (q + 1) * QS, :],
                        idxs_ap=idx_sb[:, gi * S16 : (gi + 1) * S16],
                        num_idxs=G,
                        num_idxs_reg=G,
                        elem_size=2 * D,
                        queue_num=q,
                    )
                selt = gpool.tile([P, NQ * G], f16, tag="selt")
                nc.sync.dma_start(selt[:], selt_d[w * P : (w + 1) * P, :])
                scorew = wpool.tile([P, TPW, R], f32, tag="scw")
                for half in range(TPW // 8):
                    b8 = wpool.tile([P, 8, R, D], f16, tag="b8")
                    for sub in range(2):
                        yv_ps = zpool.tile([P, 4, 512], f32, tag="yv")
                        for i in range(4):
                            t = half * 8 + sub * 4 + i
                            nc.tensor.matmul(
                                yv_ps[:, i, 0:RD],
                                lhsT=selt[:, t * P : (t + 1) * P],
                                rhs=ysb[:, w, :],
                            )
                        yv_h = wpool.tile([P, 4, RD], f16, tag="yvh")
                        nc.scalar.activation(yv_h[:], yv_ps[:, :, 0:RD], AF.Copy)
                        t0 = half * 8 + sub * 4
                        nc.vector.tensor_mul(
                            b8[:, sub * 4 : (sub + 1) * 4, :, :],
                            us_f[:, t0 : t0 + 4, 0:D]
                            .rearrange("p t (o d) -> p t o d", o=1)
                            .to_broadcast([P, 4, R, D]),
                            yv_h[:].rearrange("p t (r d) -> p t r d", r=R),
                        )
                    t32 = wpool.tile([P, 8, R, 32], f16, tag="t32")
                    nc.vector.tensor_add(t32[:], b8[:, :, :, 0:32], b8[:, :, :, 32:64])
                    t16 = wpool.tile([P, 8, R, 16], f16, tag="t16")
                    nc.vector.tensor_add(t16[:], t32[:, :, :, 0:16], t32[:, :, :, 16:32])
                    t8 = wpool.tile([P, 8, R, 8], f16, tag="t8")
                    nc.vector.tensor_add(t8[:], t16[:, :, :, 0:8], t16[:, :, :, 8:16])
                    t4 = wpool.tile([P, 8, R, 4], f32, tag="t4")
                    nc.vector.tensor_add(t4[:], t8[:, :, :, 0:4], t8[:, :, :, 4:8])
                    nc.vector.tensor_reduce(
                        out=scorew[:, half * 8 : (half + 1) * 8, :],
                        in_=t4[:],
                        axis=AX.X,
                        op=ALU.add,
                    )
                e_t = wpool.tile([P, TPW * R], f32, tag="et")
                nc.scalar.activation(
                    e_t[:], scorew[:].rearrange("p t r -> p (t r)"), AF.Exp
                )
                den = wpool.tile([P, TPW], f32, tag="den")
                nc.vector.tensor_reduce(
                    out=den[:],
                    in_=e_t[:].rearrange("p (t r) -> p t r", r=R),
                    axis=AX.X,
                    op=ALU.add,
                )
                nums = wpool.tile([P, TPW * R], f32, tag="nums")
                vals_bc = (
                    vals_sb[:]
                    .rearrange("p (o r) -> p o r", o=1)
                    .to_broadcast([P, TPW, R])
                )
                nc.vector.tensor_mul(
                    nums[:].rearrange("p (t r) -> p t r", r=R),
                    e_t[:].rearrange("p (t r) -> p t r", r=R),
                    vals_bc,
                )
                num = wpool.tile([P, TPW], f32, tag="num")
                nc.vector.tensor_reduce(
                    out=num[:],
                    in_=nums[:].rearrange("p (t r) -> p t r", r=R),
                    axis=AX.X,
                    op=ALU.add,
                )
                rden = wpool.tile([P, TPW], f32, tag="rden")
                nc.vector.reciprocal(rden[:], den[:])
                nc.vector.tensor_mul(
                    outbuf[:, w * TPW : (w + 1) * TPW], num[:], rden[:]
                )
            nc.sync.dma_start(out_d[:], outbuf[:])


def _build(NW, G):
    import concourse.bacc as bacc
    import concourse.mybir as mybir
    import concourse.tile as tile

    nc = bacc.Bacc(None, target_bir_lowering=False, num_swdge_queues=NQ)
    f32, f16, i16 = mybir.dt.float32, mybir.dt.float16, mybir.dt.int16
    TPW = NQ * G // P
    NG = NW * NQ
    ufeat_d = nc.dram_tensor("ufeat", [N_USERS, 2 * D], f16, kind="ExternalInput")
    ifT_d = nc.dram_tensor("ifT", [D, NW * P], f16, kind="ExternalInput")
    p2_d = nc.dram_tensor("p2", [D, RD], f16, kind="ExternalInput")
    idxu_d = nc.dram_tensor("idxu", [P, NG * (G // 16)], i16, kind="ExternalInput")
    selt_d = nc.dram_tensor("selt", [NW * P, NQ * G], f16, kind="ExternalInput")
    vals_d = nc.dram_tensor("vals", [P, R], f32, kind="ExternalInput")
    out_d = nc.dram_tensor("out", [P, NW * TPW], f32, kind="ExternalOutput")

    with tile.TileContext(nc) as tc:
        _emit(
            nc,
            tc,
            (ufeat_d, ifT_d, p2_d, idxu_d, selt_d, vals_d, out_d),
            NW,
            G,
        )
    nc.compile()
    return nc


def _install_profile_hook():
    """Make antenv.axon_hooks available so run_bass_kernel_spmd(trace=True)
    can capture NTFF profiles through the axon .so (used by test.py only)."""
    import types

    try:
        from antenv.axon_hooks import get_axon_ntff_profile_hook  # noqa: F401

        return
    except ImportError:
        pass
    import antenv
    from trn_agent_boot.trn_boot import _ntff_profile_via_ctypes

    hook = _ntff_profile_via_ctypes("/opt/axon/libaxon_pjrt.so")
    mod = types.ModuleType("antenv.axon_hooks")
    mod._hook = hook
    mod.get_axon_ntff_profile_hook = lambda: mod._hook
    mod.set_axon_ntff_profile_hook = lambda h: setattr(mod, "_hook", h)
    sys.modules["antenv.axon_hooks"] = mod
    antenv.axon_hooks = mod


def kernel(ufeat, ifeat, Ps, src, dst):
    from concourse.bass_utils import run_bass_kernel_spmd

    ufeat = np.asarray(ufeat, np.float32)
    ifeat = np.asarray(ifeat, np.float32)
    Ps = np.asarray(Ps, np.float32)
    src = np.asarray(src, np.int32)
    dst = np.asarray(dst, np.int32)

    in_maps, metas = _prepare(ufeat, ifeat, Ps, src, dst)
    NW, G = _NC_CACHE["params"]
    key = ("nc", NW, G)
    if key not in _NC_CACHE:
        _NC_CACHE[key] = _build(NW, G)
        _NC_CACHE["nc"] = _NC_CACHE[key]
    nc = _NC_CACHE[key]
    res = run_bass_kernel_spmd(nc, in_maps, core_ids=list(range(N_CORES)))
    out = np.zeros(E, np.float32)
    for c in range(N_CORES):
        o = res.results[c]["out"]  # [P, NW*TPW]
        flat = o.T.reshape(-1)  # slot-ordered
        s2e = metas[c]
        valid = s2e >= 0
        out[s2e[valid]] = flat[valid]
    return out


# revision 13
# speedup vs baseline: 4.5123x; 1.0208x over previous
"""TRN2 Bass kernel for nn_BiDecoder (GNN edge rating decoder), 8 NeuronCores.

ratings[e] = sum_r softmax_r(ufeat[src[e]] @ Ps[r] @ ifeat[dst[e]]) * (r+1)

v2 design:
  - Edges dst-sorted into 8 contiguous shards (each core owns an item band).
  - Within a core, items are covered by NW aligned 128-item windows; edges are
    grouped by (window w, src-quarter q) and sorted by dst inside each group.
    Each group gets G slots (G multiple of 256); trailing pads idx=-1.
  - Item side needs NO gather: Y[j, (r,d)] = sum_f Ps[r,d,f] ifeat[j,f] is
    precomputed per window on PE (fp16); per tile a one-hot SelT (built with
    tensor_mask_reduce run-intervals, since dst-sorted edges give each item a
    contiguous slot run) expands Y rows to edges via one 320-col matmul.
  - User side: dma_gather of ufeat rows, 4 src-quarters (int16 idx < 25000)
    issued round-robin on 4 SWDGE queues so descriptor generation runs on all
    Q7 cpu pairs in parallel.
  - scores = sum_d us*yv via fp16 DVE mul (2x) + binary-tree adds (2x),
    softmax (no max-sub; scores bounded) -> ratings.
"""
import sys

sys.path.insert(0, "/opt/trn_rl_repo")
import numpy as np

P = 128
D = 64
R = 5
RD = R * D
N_USERS, N_ITEMS, E = 100000, 50000, 1000000
N_CORES = 8
E_CORE = E // N_CORES
NQ = 4
QS = N_USERS // NQ  # 25000

_NC_CACHE = {}


def _plan(src, dst):
    """Shard + group edges; derive (NW, G) shared across cores."""
    perm = np.argsort(dst, kind="stable")
    shards = perm.reshape(N_CORES, E_CORE)
    cores = []
    maxw = 0
    maxcnt = 0
    for c in range(N_CORES):
        eid = shards[c]
        d = dst[eid].astype(np.int64)
        s = src[eid].astype(np.int64)
        d_lo = int(d[0])
        nw = -(-(int(d[-1]) - d_lo + 1) // P)
        maxw = max(maxw, nw)
        w = (d - d_lo) >> 7
        q = s // QS
        order = np.lexsort((d, q, w))
        eid, d, s, w, q = eid[order], d[order], s[order], w[order], q[order]
        g = w * NQ + q
        cnt = np.bincount(g, minlength=nw * NQ)
        maxcnt = max(maxcnt, int(cnt.max()))
        cores.append((eid, d, s, w, q, g, d_lo, nw, cnt))
    NW = maxw
    G = max(256, -(-maxcnt // 256) * 256)
    return cores, NW, G


def _prepare(ufeat, ifeat, Ps, src, dst):
    ufeat = np.asarray(ufeat, np.float32)
    ifeat = np.asarray(ifeat, np.float32)
    Ps = np.asarray(Ps, np.float32)
    src = np.asarray(src)
    dst = np.asarray(dst)
    cores, NW, G = _plan(src, dst)
    _NC_CACHE["params"] = (NW, G)
    S16 = G // 16
    NG = NW * NQ
    p2 = np.ascontiguousarray(Ps.transpose(2, 0, 1).reshape(D, RD)).astype(np.float16)
    vals = np.tile(np.arange(1.0, 6.0, dtype=np.float32), (P, 1))
    uf = np.zeros((N_USERS, 2 * D), np.float16)
    uf[:, :D] = ufeat.astype(np.float16)
    in_maps, metas = [], []
    for (eid, d, s, w, q, g, d_lo, nw, cnt) in cores:
        nslot = NG * G
        starts = np.zeros(NG + 1, np.int64)
        starts[1 : len(cnt) + 1] = np.cumsum(cnt)
        pos = np.arange(E_CORE) - starts[g]
        slot = g * G + pos
        idxu = np.zeros(nslot, np.int16)
        idxu[slot] = (s - q * QS).astype(np.int16)
        slot2eid = np.full(nslot, -1, np.int64)
        slot2eid[slot] = eid
        dl = d - d_lo - (w << 7)  # 0..127 within window
        # host-built one-hot SelT: [row j, slot] = 1 iff slot's dst-local == j
        selt_h = np.zeros((P, nslot), np.float16)
        selt_h[dl, slot] = 1.0
        wrapped = (
            idxu.reshape(NG, S16, 16).transpose(2, 0, 1).reshape(16, NG * S16)
        )  # [16, (group-major, col-minor)]; element i of group = [i%16, i//16]
        wrapped = np.tile(wrapped, (8, 1)).astype(np.int16)
        band = np.zeros((NW * P, D), np.float32)
        navail = min(NW * P, N_ITEMS - d_lo)
        band[:navail] = ifeat[d_lo : d_lo + navail]
        ifT = np.ascontiguousarray(band.T).astype(np.float16)
        in_maps.append(
            {
                "ufeat": uf,
                "ifT": ifT,
                "p2": p2,
                "idxu": wrapped,
                "selt": np.ascontiguousarray(
                    selt_h.reshape(P, NW, NQ * G).transpose(1, 0, 2).reshape(
                        NW * P, NQ * G
                    )
                ),
                "vals": vals,
            }
        )
        metas.append(slot2eid)
    return in_maps, metas


def _emit(nc, tc, aps, NW, G):
    import concourse.mybir as mybir
    from concourse import library_config

    f32, f16, i16 = mybir.dt.float32, mybir.dt.float16, mybir.dt.int16
    AF = mybir.ActivationFunctionType
    ALU = mybir.AluOpType
    AX = mybir.AxisListType
    TPW = NQ * G // P
    NG = NW * NQ
    S16 = G // 16
    GT = G // P  # tiles per group
    ufeat_d, ifT_d, p2_d, idxu_d, selt_d, vals_d, out_d = aps

    nc.gpsimd.load_library(library_config.mlp)
    with tc.tile_pool(name="const", bufs=1) as cpool:
        p2_sb = cpool.tile([D, RD], f16)
        nc.sync.dma_start(p2_sb[:], p2_d[:])
        ifT_sb = cpool.tile([D, NW * P], f16)
        nc.sync.dma_start(ifT_sb[:], ifT_d[:])
        idx_sb = cpool.tile([P, NG * S16], i16)
        nc.sync.dma_start(idx_sb[:], idxu_d[:])
        vals_sb = cpool.tile([P, R], f32)
        nc.sync.dma_start(vals_sb[:], vals_d[:])
        ysb = cpool.tile([P, NW, RD], f16)
        outbuf = cpool.tile([P, NW * TPW], f32)

        with tc.tile_pool(name="ypsum", bufs=2, space="PSUM") as ypool:
            for w in range(NW):
                y_ps = ypool.tile([P, 512], f32, tag="y")
                nc.tensor.matmul(
                    y_ps[:, 0:RD], lhsT=ifT_sb[:, w * P : (w + 1) * P], rhs=p2_sb[:]
                )
                nc.scalar.activation(ysb[:, w, :], y_ps[:, 0:RD], AF.Copy)

        with (
            tc.tile_pool(name="gather", bufs=3) as gpool,
            tc.tile_pool(name="work", bufs=2) as wpool,
            tc.tile_pool(name="psum_yv", bufs=2, space="PSUM") as zpool,
        ):
            for w in range(NW):
                us_f = gpool.tile([P, TPW, 2 * D], f16, tag="usf")
                for q in range(NQ):
                    gi = w * NQ + q
                    nc.gpsimd.dma_gather(
                        out_ap=us_f[:, q * GT : (q + 1) * GT, :],
                        in_ap=ufeat_d[q * QS : (q + 1) * QS, :],
                        idxs_ap=idx_sb[:, gi * S16 : (gi + 1) * S16],
                        num_idxs=G,
                        num_idxs_reg=G,
                        elem_size=2 * D,
                        queue_num=q,
                    )
                selt = gpool.tile([P, NQ * G], f16, tag="selt")
                nc.sync.dma_start(selt[:], selt_d[w * P : (w + 1) * P, :])
                scorew = wpool.tile([P, TPW, R], f32, tag="scw")
                b24 = wpool.tile([P, TPW, R, D], f16, tag="b24")
                for sub in range(TPW // 4):
                    yv_ps = zpool.tile([P, 4, 512], f32, tag="yv")
                    for i in range(4):
                        t = sub * 4 + i
                        nc.tensor.matmul(
                            yv_ps[:, i, 0:RD],
                            lhsT=selt[:, t * P : (t + 1) * P],
                            rhs=ysb[:, w, :],
                        )
                    yv_h = wpool.tile([P, 4, RD], f16, tag="yvh")
                    nc.scalar.activation(yv_h[:], yv_ps[:, :, 0:RD], AF.Copy)
                    t0 = sub * 4
                    nc.vector.tensor_mul(
                        b24[:, t0 : t0 + 4, :, :],
                        us_f[:, t0 : t0 + 4, 0:D]
                        .rearrange("p t (o d) -> p t o d", o=1)
                        .to_broadcast([P, 4, R, D]),
                        yv_h[:].rearrange("p t (r d) -> p t r d", r=R),
                    )
                t32 = wpool.tile([P, TPW, R, 32], f16, tag="t32")
                nc.vector.tensor_add(t32[:], b24[:, :, :, 0:32], b24[:, :, :, 32:64])
                t16 = wpool.tile([P, TPW, R, 16], f16, tag="t16")
                nc.vector.tensor_add(t16[:], t32[:, :, :, 0:16], t32[:, :, :, 16:32])
                t8 = wpool.tile([P, TPW, R, 8], f16, tag="t8")
                nc.vector.tensor_add(t8[:], t16[:, :, :, 0:8], t16[:, :, :, 8:16])
                t4 = wpool.tile([P, TPW, R, 4], f32, tag="t4")
                nc.vector.tensor_add(t4[:], t8[:, :, :, 0:4], t8[:, :, :, 4:8])
                nc.vector.tensor_reduce(
                    out=scorew[:], in_=t4[:], axis=AX.X, op=ALU.add
                )
                e_t = wpool.tile([P, TPW * R], f32, tag="et")
                nc.scalar.activation(
                    e_t[:], scorew[:].rearrange("p t r -> p (t r)"), AF.Exp
                )
                den = wpool.tile([P, TPW], f32, tag="den")
                nc.vector.tensor_reduce(
                    out=den[:],
                    in_=e_t[:].rearrange("p (t r) -> p t r", r=R),
                    axis=AX.X,
                    op=ALU.add,
                )
                nums = wpool.tile([P, TPW * R], f32, tag="nums")
                vals_bc = (
                    vals_sb[:]
                    .rearrange("p (o r) -> p o r", o=1)
                    .to_broadcast([P, TPW, R])
                )
                nc.vector.tensor_mul(
                    nums[:].rearrange("p (t r) -> p t r", r=R),
                    e_t[:].rearrange("p (t r) -> p t r", r=R),
                    vals_bc,
                )
                num = wpool.tile([P, TPW], f32, tag="num")
                nc.vector.tensor_reduce(
                    out=num[:],
                    in_=nums[:].rearrange("p (t r) -> p t r", r=R),
                    axis=AX.X,
                    op=ALU.add,
                )
                rden = wpool.tile([P, TPW], f32, tag="rden")
                nc.vector.reciprocal(rden[:], den[:])
                nc.vector.tensor_mul(
                    outbuf[:, w * TPW : (w + 1) * TPW], num[:], rden[:]
                )
            nc.sync.dma_start(out_d[:], outbuf[:])


def _build(NW, G):
    import concourse.bacc as bacc
    import concourse.mybir as mybir
    import concourse.tile as tile

    nc = bacc.Bacc(None, target_bir_lowering=False, num_swdge_queues=NQ)
    f32, f16, i16 = mybir.dt.float32, mybir.dt.float16, mybir.dt.int16
    TPW = NQ * G // P
    NG = NW * NQ
    ufeat_d = nc.dram_tensor("ufeat", [N_USERS, 2 * D], f16, kind="ExternalInput")
    ifT_d = nc.dram_tensor("ifT", [D, NW * P], f16, kind="ExternalInput")
    p2_d = nc.dram_tensor("p2", [D, RD], f16, kind="ExternalInput")
    idxu_d = nc.dram_tensor("idxu", [P, NG * (G // 16)], i16, kind="ExternalInput")
    selt_d = nc.dram_tensor("selt", [NW * P, NQ * G], f16, kind="ExternalInput")
    vals_d = nc.dram_tensor("vals", [P, R], f32, kind="ExternalInput")
    out_d = nc.dram_tensor("out", [P, NW * TPW], f32, kind="ExternalOutput")

    with tile.TileContext(nc) as tc:
        _emit(
            nc,
            tc,
            (ufeat_d, ifT_d, p2_d, idxu_d, selt_d, vals_d, out_d),
            NW,
            G,
        )
    nc.compile()
    return nc


def _install_profile_hook():
    """Make antenv.axon_hooks available so run_bass_kernel_spmd(trace=True)
    can capture NTFF profiles through the axon .so (used by test.py only)."""
    import types

    try:
        from antenv.axon_hooks import get_axon_ntff_profile_hook  # noqa: F401

        return
    except ImportError:
        pass
    import antenv
    from trn_agent_boot.trn_boot import _ntff_profile_via_ctypes

    hook = _ntff_profile_via_ctypes("/opt/axon/libaxon_pjrt.so")
    mod = types.ModuleType("antenv.axon_hooks")
    mod._hook = hook
    mod.get_axon_ntff_profile_hook = lambda: mod._hook
    mod.set_axon_ntff_profile_hook = lambda h: setattr(mod, "_hook", h)
    sys.modules["antenv.axon_hooks"] = mod
    antenv.axon_hooks = mod


def kernel(ufeat, ifeat, Ps, src, dst):
    from concourse.bass_utils import run_bass_kernel_spmd

    ufeat = np.asarray(ufeat, np.float32)
    ifeat = np.asarray(ifeat, np.float32)
    Ps = np.asarray(Ps, np.float32)
    src = np.asarray(src, np.int32)
    dst = np.asarray(dst, np.int32)

    in_maps, metas = _prepare(ufeat, ifeat, Ps, src, dst)
    NW, G = _NC_CACHE["params"]
    key = ("nc", NW, G)
    if key not in _NC_CACHE:
        _NC_CACHE[key] = _build(NW, G)
        _NC_CACHE["nc"] = _NC_CACHE[key]
    nc = _NC_CACHE[key]
    res = run_bass_kernel_spmd(nc, in_maps, core_ids=list(range(N_CORES)))
    out = np.zeros(E, np.float32)
    for c in range(N_CORES):
        o = res.results[c]["out"]  # [P, NW*TPW]
        flat = o.T.reshape(-1)  # slot-ordered
        s2e = metas[c]
        valid = s2e >= 0
        out[s2e[valid]] = flat[valid]
    return out
